# revision 6
# baseline (speedup 1.0000x reference)
"""Trainium2 Bass kernel for a 2-layer dense GCN (NodeEncoder).

    out = adj @ relu(adj @ (x@W1) + b1) @ W2 + b2
    N=16384, F_IN=512, HID=1024, OUT=256, adj dense [N, N] fp32.

Key algebraic optimization vs the straightforward lowering: layer 1 is
computed as (adj @ x) @ W1 instead of adj @ (x @ W1).  The adj
contraction then runs against F_IN=512 columns instead of HID=1024,
halving the dominant matmul's FLOPs (275 vs 550 GFLOP), and since x is
replicated on every core the layer-1 AllGather disappears entirely.

Sharding: adj row-partitioned across 8 NeuronCores (2048 rows/core).
Per core (all matmuls bf16 with fp32 PSUM accumulation):

  phase A:  zT_c   = (adj_c @ x)^T          [512, 2048]   (lhsT = x
            blocks stationary, rhs = adjT_c streaming; out is zT)
  phase H:  hT_c   = relu(z_c @ W1 + b1)^T  [1024, 2048]  (lhsT = W1
            blocks, rhs = zT tiles; bias per-partition in ACT relu)
  phase S:  s2_c   = h_c @ W2               [2048, 256]   (lhsT = hT
            blocks, rhs = W2)
  AG:       s2     = AllGather(s2_c)        [16384, 256]  (in quarters,
            fired as soon as each quarter of s2_c is ready)
  phase D:  out2T_c = (adj_c @ s2)^T + b2   [256, 2048]   (lhsT = s2
            tiles, rhs = adjT_c streaming; b2 via ACT Identity)

Phases A/H/S are split in two m-chunks (1024 adj columns each) so the
first two AG quarters fire halfway through phase A and the gather
overlaps compute; phase D consumes k-blocks in gather-arrival order.
"""

import numpy as np
import ml_dtypes

import concourse.bass as bass
import concourse.mybir as mybir
import concourse.tile as tile
from concourse.bass_utils import run_bass_kernel_spmd
from concourse.tile_sem_assignment import N_PROCS
from concourse.vector_clock import ScopedClock, VectorClock

# ---------------------------------------------------------------------------
# Workaround: the walrus build in this container caps the number of sync-wait
# commands on a Drain instruction; Tile's kernel-tail drain aggregates one
# wait per logical processor and exceeds it.  Split the tail drain into a
# chain of single-wait drains on the same (SP) queue — semantically identical.
# ---------------------------------------------------------------------------


def _drain_and_barrier_split(self, tick_clock, wait_clock):
    gc = tick_clock.global_clock
    for p in range(N_PROCS):
        partial = VectorClock([gc[q] if q == p else 0 for q in range(N_PROCS)])
        d = self.nc.sync.drain()
        wait_clock.add_sem_waits(d.ins, ScopedClock({None: partial}))
    self.nc.sync.drain()

    self.nc.all_engine_barrier()
    assert self.sems is not None
    popped = self.nc._tile_sem_poison_stack.pop()
    assert popped is self._sem_poison
    self.nc.clear_and_free_semaphores(list(self.sems.allocated().values()))
    self.nc.all_engine_barrier()


tile.TileContext._drain_and_barrier = _drain_and_barrier_split

# The same walrus cap applies to every instruction kind: at most ONE sync
# wait command per instruction (probed empirically — a 2-wait TensorCopy is
# rejected).  Post-pass: hoist excess sem-waits onto no-ops inserted just
# before the instruction on the same engine queue — per-engine program order
# makes this semantically identical.
_MAX_WAITS = 1


def _split_excess_waits(nc):
    ctr = 0
    for f in nc.m.functions:
        for bb in f.blocks:
            out = []
            changed = False
            for inst in bb.instructions:
                si = inst.sync_info
                waits = list(si.on_wait) if si is not None and si.on_wait else []
                if len(waits) > _MAX_WAITS:
                    changed = True
                    keep, excess = waits[: _MAX_WAITS], waits[_MAX_WAITS :]
                    for i in range(0, len(excess), _MAX_WAITS):
                        ctr += 1
                        nop = mybir.InstNoOp(name=f"I-waitnop-{ctr}")
                        nop.engine = inst.engine
                        nop.sync_info = mybir.SyncInfo(
                            on_wait=excess[i : i + _MAX_WAITS], on_update=[]
                        )
                        out.append(nop)
                    si.on_wait = keep
                out.append(inst)
            if changed:
                bb.instructions = out
    return ctr


def _elide_redundant_ldweights(nc):
    """Delete an InstLdweights that reloads the exact weights AP loaded by
    the previous (surviving) InstLdweights when only plain matmuls / no-ops
    sit between them in the scheduled stream.  The PE array keeps the
    stationary operand across matmuls, so the reload is pure overhead
    (walrus emits one LDWEIGHTS per MATMUL and its ldw-opt pass is
    incompatible with pre-split LDW+MM).  Only sync-free LDWs are removed,
    so semaphore bookkeeping is unchanged."""
    n_elided = 0
    for f in nc.m.functions:
        for bb in f.blocks:
            out = []
            last_w = None  # weights-AP repr of last surviving LDW, if run intact
            changed = False
            for inst in bb.instructions:
                nm = type(inst).__name__
                if nm == "InstLdweights":
                    si = inst.sync_info
                    clean = not (si and (si.on_wait or si.on_update))
                    w = repr(inst.ins[0])
                    if clean and last_w == w:
                        n_elided += 1
                        changed = True
                        continue  # drop the reload
                    last_w = w if clean else None
                elif nm == "InstMatmult":
                    if getattr(inst, "is_transpose", False):
                        last_w = None
                elif nm == "InstNoOp":
                    pass
                else:
                    last_w = None
                out.append(inst)
            if changed:
                bb.instructions = out
    return n_elided


NCORES = 8
N = 16384
SH = N // NCORES  # 2048 adj rows per core
F = 512
HID = 1024
OUT = 256

BF16 = mybir.dt.bfloat16
F32 = mybir.dt.float32

_built = None


def build():
    """Build the per-core Bass program (identical on all cores)."""
    nc = bass.Bass()

    adjT = nc.declare_dram_parameter("adjT", [N, SH], BF16, isOutput=False)
    xfull = nc.declare_dram_parameter("xfull", [N, F], BF16, isOutput=False)
    w1 = nc.declare_dram_parameter("w1", [F, HID], BF16, isOutput=False)
    w2 = nc.declare_dram_parameter("w2", [HID, OUT], BF16, isOutput=False)
    b1T = nc.declare_dram_parameter("b1T", [128, HID // 128], F32, isOutput=False)
    b2T = nc.declare_dram_parameter("b2T", [128, OUT // 128], F32, isOutput=False)
    out2T = nc.declare_dram_parameter("out2T", [OUT, SH], F32, isOutput=True)

    rg = [list(range(NCORES))]

    def allgather(inp, outp):
        return nc.gpsimd.collective_compute(
            "AllGather",
            mybir.AluOpType.bypass,
            replica_groups=rg,
            ins=[inp.opt()],
            outs=[outp.opt()],
        )

    with tile.TileContext(nc) as tc:
        with (
            tc.tile_pool(name="const", bufs=1) as constp,
            tc.tile_pool(name="psum", bufs=8, space="PSUM") as psum,
            tc.tile_pool(name="dram", bufs=1, space="DRAM") as dram,
        ):
            # ---- constants (ACT HWDGE ring; adj streams ride the SP ring) --
            w1t = constp.tile([128, F // 128, HID], BF16)
            nc.scalar.dma_start(w1t[:], w1[:].rearrange("(fb p) j -> p fb j", p=128))
            w2t = constp.tile([128, HID // 128, OUT], BF16)
            nc.scalar.dma_start(w2t[:], w2[:].rearrange("(jb p) n -> p jb n", p=128))
            b1t = constp.tile([128, HID // 128], F32)
            nc.scalar.dma_start(b1t[:], b1T[:])
            b2t = constp.tile([128, OUT // 128], F32)
            nc.scalar.dma_start(b2t[:], b2T[:])

            ag_in = [dram.tile([SH // 4, OUT], BF16, name=f"agi{q}") for q in range(4)]
            ag_out = [
                dram.tile([N // 4, OUT], BF16, addr_space="Shared", name=f"ago{q}")
                for q in range(4)
            ]

            # x replicated: [128, kb, f] with node = kb*128 + p
            xsrc = xfull[:].rearrange("(kb p) f -> p kb f", p=128)

            with (
                tc.tile_pool(name="xp", bufs=1) as xp,
                tc.tile_pool(name="zt", bufs=8) as ztp,
                tc.tile_pool(name="ht", bufs=16) as htp,
                tc.tile_pool(name="adjA", bufs=3) as adjp,
                tc.tile_pool(name="small", bufs=4) as smallp,
            ):
                xts = []  # 8 tiles of 16 k-blocks each
                zt = {}
                ht = {}
                for c in range(2):
                    # adjT column-chunk (1024 wide), 4 k-blocks per DMA:
                    #   [p, k4, kk, m] = adjT[k4*512 + kk*128 + p, c*1024 + m]
                    asrc = adjT[:, c * 1024 : (c + 1) * 1024].rearrange(
                        "(k4 kk p) m -> p k4 kk m", kk=4, p=128
                    )
                    # ---- phase A: zT chunk = (adj_c @ x)^T cols c*1024.. ----
                    ps = [
                        psum.tile([128, 512], F32, tag="ps", name=f"psA{c}{i}")
                        for i in range(8)
                    ]
                    for k4 in range(32):
                        if c == 0 and k4 % 4 == 0:
                            i = k4 // 4
                            t = xp.tile([128, 16, F], BF16, name=f"xt{i}")
                            nc.scalar.dma_start(t[:], xsrc[:, i * 16 : (i + 1) * 16])
                            xts.append(t)
                        at = adjp.tile(
                            [128, 4, 1024], BF16, tag="adjA", bufs=3, name=f"aA{c}{k4}"
                        )
                        nc.sync.dma_start(at[:], asrc[:, k4])
                        for kk in range(4):
                            kb = k4 * 4 + kk
                            for fb in range(4):
                                lhs = xts[kb // 16][:, kb % 16, fb * 128 : (fb + 1) * 128]
                                for mh in range(2):
                                    nc.tensor.matmul(
                                        ps[fb * 2 + mh][:],
                                        lhs,
                                        at[:, kk, mh * 512 : (mh + 1) * 512],
                                        start=(kb == 0),
                                        stop=(kb == 127),
                                    )
                    for fb in range(4):
                        for mh in range(2):
                            zz = ztp.tile(
                                [128, 512], BF16, tag="zt", bufs=8, name=f"zt{c}{fb}{mh}"
                            )
                            nc.vector.tensor_copy(zz[:], ps[fb * 2 + mh][:])
                            zt[(fb, c * 2 + mh)] = zz

                    # ---- phase H: hT chunk = relu(z @ W1 + b1)^T ----
                    for jbh in range(2):
                        psh = [
                            psum.tile([128, 512], F32, tag="ps", name=f"psH{c}{jbh}{i}")
                            for i in range(8)
                        ]
                        for jb in range(4):
                            jg = jbh * 4 + jb
                            for fb in range(4):
                                lhs = w1t[:, fb, jg * 128 : (jg + 1) * 128]
                                for mh in range(2):
                                    nc.tensor.matmul(
                                        psh[jb * 2 + mh][:],
                                        lhs,
                                        zt[(fb, c * 2 + mh)][:],
                                        start=(fb == 0),
                                        stop=(fb == 3),
                                    )
                            for mh in range(2):
                                hh = htp.tile(
                                    [128, 512], BF16, tag="ht", bufs=16,
                                    name=f"ht{c}{jbh}{jb}{mh}",
                                )
                                nc.scalar.activation(
                                    hh[:],
                                    psh[jb * 2 + mh][:],
                                    mybir.ActivationFunctionType.Relu,
                                    bias=b1t[:, jg : jg + 1],
                                )
                                ht[(jg, c * 2 + mh)] = hh

                    # ---- phase S: s2 chunk = h @ W2; AG per quarter ----
                    for qq in (c * 2, c * 2 + 1):
                        for m4 in range(4):
                            pss = psum.tile([128, 256], F32, tag="ps", name=f"psS{qq}{m4}")
                            for jb in range(8):
                                nc.tensor.matmul(
                                    pss[:],
                                    ht[(jb, qq)][:, m4 * 128 : (m4 + 1) * 128],
                                    w2t[:, jb, :],
                                    start=(jb == 0),
                                    stop=(jb == 7),
                                )
                            so = smallp.tile([128, 256], BF16, tag="so", bufs=2)
                            nc.vector.tensor_copy(so[:], pss[:])
                            nc.scalar.dma_start(
                                ag_in[qq][m4 * 128 : (m4 + 1) * 128, :], so[:]
                            )
                        allgather(ag_in[qq], ag_out[qq])

            # ---- phase D: out2T = (adj_c @ s2)^T + b2 ----
            # All 8 psum banks accumulate concurrently; k-blocks consumed in
            # gather-arrival order (quarter-major), s2 tiles loaded JIT after
            # each adjT chunk so the SP queue stays load-ordered.
            with (
                tc.tile_pool(name="adjD", bufs=3) as adjDp,
                tc.tile_pool(name="s2p", bufs=4) as s2p,
                tc.tile_pool(name="outp", bufs=8) as outp,
            ):
                # ag_out[qq] rows = g*512 + skk*128 + p  (rank g, block qq)
                s2srcs = [
                    ag_out[qq][:].rearrange("(g skk p) n -> p g skk n", g=8, p=128)
                    for qq in range(4)
                ]
                dsrc = adjT[:].rearrange("(k4 kk p) m -> p k4 kk m", kk=4, p=128)
                dps = [
                    psum.tile([128, 512], F32, tag="ps", name=f"psD{i}")
                    for i in range(8)
                ]
                # k4 = g*4 + qq  ->  iterate quarter-major
                k4_order = [g * 4 + qq for qq in range(4) for g in range(8)]
                for ki, k4 in enumerate(k4_order):
                    g, qq = k4 // 4, k4 % 4
                    at = adjDp.tile([128, 4, SH], BF16, tag="adjD", name=f"aD{k4}")
                    nc.sync.dma_start(at[:], dsrc[:, k4])
                    st = s2p.tile([128, 4, OUT], BF16, tag="s2t", name=f"s2t{k4}")
                    nc.sync.dma_start(st[:], s2srcs[qq][:, g])
                    for kk in range(4):
                        for n2t in range(2):
                            lhs = st[:, kk, n2t * 128 : (n2t + 1) * 128]
                            for mb in range(4):
                                nc.tensor.matmul(
                                    dps[n2t * 4 + mb][:],
                                    lhs,
                                    at[:, kk, mb * 512 : (mb + 1) * 512],
                                    start=(ki == 0 and kk == 0),
                                    stop=(ki == 31 and kk == 3),
                                )
                for n2t in range(2):
                    for mb in range(4):
                        ot = outp.tile([128, 512], F32, tag="ot")
                        nc.scalar.activation(
                            ot[:],
                            dps[n2t * 4 + mb][:],
                            mybir.ActivationFunctionType.Identity,
                            bias=b2t[:, n2t : n2t + 1],
                        )
                        nc.scalar.dma_start(
                            out2T[
                                n2t * 128 : (n2t + 1) * 128, mb * 512 : (mb + 1) * 512
                            ],
                            ot[:],
                        )

    _elide_redundant_ldweights(nc)
    _split_excess_waits(nc)
    return nc


def _prep_inputs(x, adj, W1, b1, W2, b2):
    bf = ml_dtypes.bfloat16
    xb = np.ascontiguousarray(x).astype(bf)
    w1b = W1.astype(bf)
    w2b = W2.astype(bf)
    b1T = np.ascontiguousarray(b1.reshape(HID // 128, 128).T).astype(np.float32)
    b2T = np.ascontiguousarray(b2.reshape(OUT // 128, 128).T).astype(np.float32)
    in_maps = []
    for c in range(NCORES):
        rows = slice(c * SH, (c + 1) * SH)
        in_maps.append(
            {
                "adjT": adj[rows, :].T.astype(bf),
                "xfull": xb,
                "w1": w1b,
                "w2": w2b,
                "b1T": b1T,
                "b2T": b2T,
            }
        )
    return in_maps


def _run(inputs, trace=False):
    global _built
    if _built is None:
        _built = build()
    in_maps = _prep_inputs(**inputs)
    r = run_bass_kernel_spmd(_built, in_maps, list(range(NCORES)), trace=trace)
    out = np.empty([N, OUT], np.float32)
    for c in range(NCORES):
        out[c * SH : (c + 1) * SH, :] = r.results[c]["out2T"].T
    return out, r


def kernel(x, adj, W1, b1, W2, b2):
    out, _ = _run(dict(x=x, adj=adj, W1=W1, b1=b1, W2=W2, b2=b2))
    return out
